# revision 1
# baseline (speedup 1.0000x reference)
"""Trainium2 Bass kernel for the GNN bi-interaction aggregator.

side = segment_sum(ego[edge_cols] * edge_vals, edge_rows)
out  = leaky_relu((ego + side) @ W1.T + b1) + leaky_relu((ego * side) @ W2.T + b2)

Sharding: destination nodes (rows) split across 8 NeuronCores; the
embedding table is replicated in bf16 for the edge gather.  Each core runs
its own compiled program (the edge structure is baked in statically):
  - SWDGE dma_gather fetches neighbor rows (bf16, 512B each) for 128-edge
    tiles; the int16 index limit is handled by splitting the table into
    four <=25000-row chunks and grouping each gather call by chunk.
  - A sparse selector matrix S (DVE: iota==dest compares scaled by edge
    vals) turns segment-sum into PE matmuls accumulating into a PSUM
    block of 128 destinations; non-first tiles use 64-wide dest windows.
  - The bi-interaction MLP runs on PE (transposes + two 256x256 matmuls),
    leaky-relu and final add on DVE.
"""
import sys
import threading

import numpy as np

if "/opt/trn_rl_repo" not in sys.path:
    sys.path.append("/opt/trn_rl_repo")  # fallback when axon _ro copy absent

import ml_dtypes  # noqa: E402
import concourse.bass as bass  # noqa: E402
import concourse.bacc as bacc  # noqa: E402
import concourse.mybir as mybir  # noqa: E402
from concourse.tile import TileContext  # noqa: E402
from concourse.masks import make_identity  # noqa: E402

P = 128
W = 64              # dest window width (PSUM matmul base partition: 0 or 64)
D = 256
N_CORES = 8
CHUNK = 25000       # table chunk rows (int16 gather index limit)
MAX_CALL_TILES = 16  # <=2048 idxs per dma_gather call
PAIR = 2            # blocks per gather group
F32 = mybir.dt.float32
BF16 = mybir.dt.bfloat16
I16 = mybir.dt.int16
AL = mybir.AluOpType
NEG_SLOPE = 0.01
_LAST_RUNNERS = []
_LAST_NCS = []


# ---------------- host preprocessing ----------------

def _window_tiles(rb, cb, vb):
    """Split dest-sorted edges of one (block, chunk) list into W-window
    tiles of <=128 edges. Returns list of (idx128, dest_rel128, val128, o)."""
    out = []
    pos = 0
    ne = len(rb)
    while pos < ne:
        o = 0 if rb[pos] < W else W
        stop = np.searchsorted(rb, o + W, side='left')
        take = min(P, stop - pos)
        t_idx = np.full(P, cb[pos], np.int64)
        t_dst = np.zeros(P, np.float32)
        t_val = np.zeros(P, np.float32)
        t_idx[:take] = cb[pos:pos + take]
        t_dst[:take] = rb[pos:pos + take] - o
        t_val[:take] = vb[pos:pos + take]
        out.append((t_idx, t_dst, t_val, o))
        pos += take
    return out


def preprocess_core(rows, cols, vals, lo, hi):
    """Static tile/gather structure for destination rows [lo, hi)."""
    sel = (rows >= lo) & (rows < hi)
    r, c, v = rows[sel] - lo, cols[sel], vals[sel]
    order = np.argsort(r, kind='stable')
    r, c, v = r[order], c[order], v[order]
    nnodes = hi - lo
    nblocks = (nnodes + P - 1) // P
    blk = r // P
    blk_starts = np.searchsorted(blk, np.arange(nblocks + 1))

    # per block: full tile + per-chunk windowed tiles
    blk_full = []          # (idx128, dest128, val128) per block
    blk_wtiles = []        # per block: dict chunk -> list of windowed tiles
    for b in range(nblocks):
        s, e = blk_starts[b], blk_starts[b + 1]
        rb, cb, vb = r[s:e] - b * P, c[s:e], v[s:e]
        ch = cb // CHUNK
        per_chunk = {}
        for cc in range(4):
            m = ch == cc
            if m.any():
                per_chunk[cc] = (rb[m], cb[m], vb[m])
        if per_chunk:
            c0 = max(per_chunk, key=lambda k: len(per_chunk[k][0]))
            rb0, cb0, vb0 = per_chunk[c0]
            n0 = min(P, len(rb0))
            t_idx = np.full(P, cb0[0], np.int64)
            t_dst = np.zeros(P, np.float32)
            t_val = np.zeros(P, np.float32)
            t_idx[:n0] = cb0[:n0]; t_dst[:n0] = rb0[:n0]; t_val[:n0] = vb0[:n0]
            full = (t_idx, t_dst, t_val, c0)
            rest = (rb0[n0:], cb0[n0:], vb0[n0:])
            if len(rest[0]):
                per_chunk[c0] = rest
            else:
                del per_chunk[c0]
        else:
            full = (np.zeros(P, np.int64), np.zeros(P, np.float32),
                    np.zeros(P, np.float32), 0)
        blk_full.append(full)
        blk_wtiles.append({cc: _window_tiles(*per_chunk[cc]) for cc in per_chunk})

    # enumerate gather groups (pairs of blocks), slots in (chunk, block) order
    idx16_cols = []        # list of [128, nidx/16] int16 arrays (per call)
    destr_l, valsr_l = [], []
    dest0_l, vals0_l = [], []
    groups = []            # per group: dict(calls=[(chunk, slot0, ntiles, icol0)], nslots, blocks=[(b, full_slot, [(slot, o)...])])
    gi = 0
    for g0 in range(0, nblocks, PAIR):
        gblocks = list(range(g0, min(g0 + PAIR, nblocks)))
        slots = []         # (b, kind, tile_data, chunk)
        for cc in range(4):
            for b in gblocks:
                fi, fd, fv, fc = blk_full[b]
                if fc == cc:
                    slots.append((b, 'full', (fi, fd, fv), cc))
                for t in blk_wtiles[b].get(cc, []):
                    slots.append((b, 'win', t, cc))
        # assign slot numbers, build calls split by chunk and MAX_CALL_TILES
        calls = []
        s0 = 0
        while s0 < len(slots):
            cc = slots[s0][3]
            s1 = s0
            while s1 < len(slots) and slots[s1][3] == cc and s1 - s0 < MAX_CALL_TILES:
                s1 += 1
            ntiles = s1 - s0
            flat = np.concatenate([slots[s][2][0] for s in range(s0, s1)])
            flat = flat - cc * CHUNK
            assert flat.min() >= 0 and flat.max() < CHUNK
            i16 = np.tile(flat.reshape(-1, 16).T.astype(np.int16), (8, 1))
            calls.append((cc, s0, ntiles, sum(x.shape[1] for x in idx16_cols)))
            idx16_cols.append(i16)
            s0 = s1
        # per-block MM info
        binfo = []
        for b in gblocks:
            full_slot = None
            wslots = []
            for si, (bb, kind, td, cc) in enumerate(slots):
                if bb != b:
                    continue
                if kind == 'full':
                    full_slot = si
                    dest0_l.append(td[1]); vals0_l.append(td[2])
                else:
                    wslots.append((si, td[3], len(destr_l)))
                    destr_l.append(td[1]); valsr_l.append(td[2])
            binfo.append((b, full_slot, wslots))
        groups.append(dict(calls=calls, nslots=len(slots), blocks=binfo))
        gi += 1

    idx16 = (np.concatenate(idx16_cols, axis=1) if idx16_cols
             else np.zeros((P, 16), np.int16))
    dest0 = np.stack(dest0_l, axis=1).astype(np.float32)
    vals0 = np.stack(vals0_l, axis=1).astype(np.float32)
    destr = (np.stack(destr_l, axis=1).astype(np.float32) if destr_l
             else np.zeros((P, 1), np.float32))
    valsr = (np.stack(valsr_l, axis=1).astype(np.float32) if valsr_l
             else np.zeros((P, 1), np.float32))
    return dict(idx16=idx16, dest0=dest0, vals0=vals0, destr=destr, valsr=valsr,
                groups=groups, nblocks=nblocks, nnodes=nnodes)


def make_core_inputs(struct, table_bf16, ego_slice, W1, b1, W2, b2):
    w1t = np.ascontiguousarray(W1.T.astype(np.float32)).reshape(2, P, D)
    w2t = np.ascontiguousarray(W2.T.astype(np.float32)).reshape(2, P, D)
    return {
        "table": table_bf16,
        "ego": np.ascontiguousarray(ego_slice.astype(np.float32)),
        "idx16": struct["idx16"],
        "dest0": struct["dest0"], "vals0": struct["vals0"],
        "destr": struct["destr"].astype(ml_dtypes.bfloat16),
        "valsr": struct["valsr"].astype(ml_dtypes.bfloat16),
        "w1t": w1t, "w2t": w2t,
        "b1": b1.astype(np.float32).reshape(1, D),
        "b2": b2.astype(np.float32).reshape(1, D),
        "ones": np.ones((1, P), np.float32),
        "iota128": np.tile(np.arange(P, dtype=ml_dtypes.bfloat16), (P, 1)),
        "iotaW": np.tile(np.arange(W, dtype=ml_dtypes.bfloat16), (P, 1)),
    }


# ---------------- program builder ----------------

def build_core_program(struct, n_table_rows, gbufs=2, reps=1, stage='full'):
    nb = struct["nblocks"]
    nnodes = struct["nnodes"]
    Ti = struct["idx16"].shape[1]
    nfull = struct["dest0"].shape[1]
    Tr = struct["destr"].shape[1]
    groups = struct["groups"]

    nc = bacc.Bacc("TRN2", target_bir_lowering=False, debug=False, num_swdge_queues=4)
    table = nc.dram_tensor("table", [n_table_rows, D], BF16, kind="ExternalInput")
    ego = nc.dram_tensor("ego", [nnodes, D], F32, kind="ExternalInput")
    idx16 = nc.dram_tensor("idx16", [P, Ti], I16, kind="ExternalInput")
    dest0 = nc.dram_tensor("dest0", [P, nfull], F32, kind="ExternalInput")
    vals0 = nc.dram_tensor("vals0", [P, nfull], F32, kind="ExternalInput")
    destr = nc.dram_tensor("destr", [P, Tr], BF16, kind="ExternalInput")
    valsr = nc.dram_tensor("valsr", [P, Tr], BF16, kind="ExternalInput")
    w1t = nc.dram_tensor("w1t", [2, P, D], F32, kind="ExternalInput")
    w2t = nc.dram_tensor("w2t", [2, P, D], F32, kind="ExternalInput")
    b1 = nc.dram_tensor("b1", [1, D], F32, kind="ExternalInput")
    b2 = nc.dram_tensor("b2", [1, D], F32, kind="ExternalInput")
    ones = nc.dram_tensor("ones", [1, P], F32, kind="ExternalInput")
    iota128 = nc.dram_tensor("iota128", [P, P], BF16, kind="ExternalInput")
    iotaW = nc.dram_tensor("iotaW", [P, W], BF16, kind="ExternalInput")
    out = nc.dram_tensor("out", [nnodes, D], F32, kind="ExternalOutput")

    with TileContext(nc) as tc:
        with (
            tc.tile_pool(name="const", bufs=1) as cpool,
            tc.tile_pool(name="g", bufs=gbufs) as gpool,
            tc.tile_pool(name="s", bufs=2) as spool,
            tc.tile_pool(name="e", bufs=2) as epool,
            tc.tile_pool(name="m", bufs=2) as mpool,
            tc.tile_pool(name="pside", bufs=3, space="PSUM") as pside_pool,
            tc.tile_pool(name="pt", bufs=2, space="PSUM") as pt_pool,
            tc.tile_pool(name="pz", bufs=2, space="PSUM") as pz_pool,
        ):
            dest0_sb = cpool.tile([P, nfull], F32)
            nc.sync.dma_start(out=dest0_sb[:], in_=dest0[:, :])
            vals0_sb = cpool.tile([P, nfull], F32)
            nc.sync.dma_start(out=vals0_sb[:], in_=vals0[:, :])
            w1t_sb = cpool.tile([P, 2, D], F32)
            nc.sync.dma_start(out=w1t_sb[:], in_=w1t[:, :, :].transpose([1, 0, 2]))
            w2t_sb = cpool.tile([P, 2, D], F32)
            nc.sync.dma_start(out=w2t_sb[:], in_=w2t[:, :, :].transpose([1, 0, 2]))
            b1_sb = cpool.tile([1, D], F32)
            nc.sync.dma_start(out=b1_sb[:], in_=b1[:, :])
            b2_sb = cpool.tile([1, D], F32)
            nc.sync.dma_start(out=b2_sb[:], in_=b2[:, :])
            ones_sb = cpool.tile([1, P], F32)
            nc.sync.dma_start(out=ones_sb[:], in_=ones[:, :])
            iota128_sb = cpool.tile([P, P], BF16)
            nc.sync.dma_start(out=iota128_sb[:], in_=iota128[:, :])
            iotaW_sb = cpool.tile([P, W], BF16)
            nc.sync.dma_start(out=iotaW_sb[:], in_=iotaW[:, :])
            ident = cpool.tile([P, P], F32)
            make_identity(nc, ident[:])

            qrr = 0
            for _rep in range(reps):
              full_i = 0  # running index into dest0/vals0 columns
              for g in groups:
                nslots = g["nslots"]
                g_icol0 = g["calls"][0][3]
                g_icols = nslots * P // 16
                idx_sb = spool.tile([P, g_icols], I16, tag="idx")
                nc.sync.dma_start(out=idx_sb[:], in_=idx16[:, g_icol0:g_icol0 + g_icols])
                # ---- gather all slots of the group
                G = gpool.tile([P, nslots, D], BF16, tag="G")
                for (cc, s0, ntiles, icol0) in g["calls"]:
                    nidx = ntiles * P
                    li = icol0 - g_icol0
                    if stage == 'densefill':
                        src_rows = table[cc * CHUNK:cc * CHUNK + nidx, :]
                        nc.sync.dma_start(
                            out=G[:, s0:s0 + ntiles, :],
                            in_=src_rows.rearrange("(p k) d -> p k d", p=P))
                    else:
                        nc.gpsimd.dma_gather(
                            out_ap=G[:, s0:s0 + ntiles, :],
                            in_ap=table[cc * CHUNK:min((cc + 1) * CHUNK, n_table_rows), :],
                            idxs_ap=idx_sb[:, li:li + nidx // 16],
                            num_idxs=nidx, num_idxs_reg=nidx, elem_size=D,
                            single_packet=False, queue_num=qrr % 4)
                        qrr += 1
                if stage == 'gather':
                    continue
                # ---- S build (full tiles individually, windowed batched)
                w0 = None
                for (b, full_slot, wslots) in g["blocks"]:
                    if w0 is None and wslots:
                        w0 = wslots[0][2]
                nwin = sum(len(ws) for (_, _, ws) in g["blocks"])
                S = spool.tile([P, P * len(g["blocks"]) + W * max(nwin, 1)],
                               BF16, tag="S")
                scol = {}
                pos = 0
                for bi, (b, full_slot, wslots) in enumerate(g["blocks"]):
                    fcol = full_i + bi
                    nc.vector.tensor_scalar(
                        out=S[:, pos:pos + P], in0=iota128_sb[:],
                        scalar1=dest0_sb[:, fcol:fcol + 1],
                        scalar2=vals0_sb[:, fcol:fcol + 1],
                        op0=AL.is_equal, op1=AL.mult)
                    scol[full_slot] = (pos, P)
                    pos += P
                if nwin:
                    destr_sb = spool.tile([P, nwin], BF16, tag="destr")
                    nc.sync.dma_start(out=destr_sb[:], in_=destr[:, w0:w0 + nwin])
                    valsr_sb = spool.tile([P, nwin], BF16, tag="valsr")
                    nc.sync.dma_start(out=valsr_sb[:], in_=valsr[:, w0:w0 + nwin])
                    M = spool.tile([P, W * nwin], BF16, tag="M")
                    i0 = iotaW_sb[:].unsqueeze(1).broadcast_to([P, nwin, W])
                    i1 = destr_sb[:, 0:nwin].unsqueeze(2).broadcast_to([P, nwin, W])
                    vv = valsr_sb[:, 0:nwin].unsqueeze(2).broadcast_to([P, nwin, W])
                    Mv = M[:].rearrange("p (t w) -> p t w", w=W)
                    Sv = S[:, pos:pos + W * nwin].rearrange("p (t w) -> p t w", w=W)
                    nc.vector.tensor_tensor(out=Mv, in0=i0, in1=i1, op=AL.is_equal)
                    nc.vector.tensor_tensor(out=Sv, in0=Mv, in1=vv, op=AL.mult)
                    wi = 0
                    for (b, full_slot, wslots) in g["blocks"]:
                        for (si, o, rcol) in wslots:
                            assert rcol == w0 + wi
                            scol[si] = (pos + W * wi, W)
                            wi += 1
                # ---- per-block SpMM + MLP
                for (b, full_slot, wslots) in g["blocks"]:
                    lo = b * P
                    nn = min(P, nnodes - lo)
                    pside = pside_pool.tile([P, D], F32, tag="side")
                    sc, _ = scol[full_slot]
                    nc.tensor.matmul(out=pside[:], lhsT=S[:, sc:sc + P],
                                     rhs=G[:, full_slot, :],
                                     start=True, stop=(not wslots),
                                     skip_group_check=True)
                    for wi, (si, o, rcol) in enumerate(wslots):
                        sc, swd = scol[si]
                        nc.tensor.matmul(out=pside[o:o + W, :],
                                         lhsT=S[:, sc:sc + swd],
                                         rhs=G[:, si, :],
                                         start=False, stop=(wi == len(wslots) - 1),
                                         skip_group_check=True)
                    E = epool.tile([P, D], F32, tag="E")
                    if nn < P:
                        nc.vector.memset(E[:], 0.0)
                    nc.sync.dma_start(out=E[:nn, :], in_=ego[lo:lo + nn, :])
                    sum_in = mpool.tile([P, D], F32, tag="sum_in")
                    nc.vector.tensor_tensor(out=sum_in[:], in0=E[:], in1=pside[:], op=AL.add)
                    bi_in = mpool.tile([P, D], F32, tag="bi_in")
                    nc.vector.tensor_tensor(out=bi_in[:], in0=E[:], in1=pside[:], op=AL.mult)
                    pT = pt_pool.tile([P, 2 * D], F32, tag="T")
                    nc.tensor.transpose(out=pT[:, 0:P], in_=sum_in[:, :P], identity=ident[:])
                    nc.tensor.transpose(out=pT[:, P:2 * P], in_=sum_in[:, P:], identity=ident[:])
                    nc.tensor.transpose(out=pT[:, 2 * P:3 * P], in_=bi_in[:, :P], identity=ident[:])
                    nc.tensor.transpose(out=pT[:, 3 * P:4 * P], in_=bi_in[:, P:], identity=ident[:])
                    sbT = mpool.tile([P, 2 * D], F32, tag="sbT")
                    nc.scalar.copy(out=sbT[:], in_=pT[:])
                    sumT = sbT[:, :D]
                    biT = sbT[:, D:]
                    pz = pz_pool.tile([P, 2 * D], F32, tag="Z")
                    nc.tensor.matmul(out=pz[:, :D], lhsT=sumT[:, :P], rhs=w1t_sb[:, 0, :],
                                     start=True, stop=False, skip_group_check=True)
                    nc.tensor.matmul(out=pz[:, :D], lhsT=sumT[:, P:D], rhs=w1t_sb[:, 1, :],
                                     start=False, stop=False, skip_group_check=True)
                    nc.tensor.matmul(out=pz[:, :D], lhsT=ones_sb[:, :], rhs=b1_sb[:, :],
                                     start=False, stop=True, skip_group_check=True)
                    nc.tensor.matmul(out=pz[:, D:], lhsT=biT[:, :P], rhs=w2t_sb[:, 0, :],
                                     start=True, stop=False, skip_group_check=True)
                    nc.tensor.matmul(out=pz[:, D:], lhsT=biT[:, P:D], rhs=w2t_sb[:, 1, :],
                                     start=False, stop=False, skip_group_check=True)
                    nc.tensor.matmul(out=pz[:, D:], lhsT=ones_sb[:, :], rhs=b2_sb[:, :],
                                     start=False, stop=True, skip_group_check=True)
                    lz = mpool.tile([P, 2 * D], F32, tag="lz")
                    nc.vector.tensor_scalar(out=lz[:], in0=pz[:], scalar1=NEG_SLOPE,
                                            scalar2=None, op0=AL.mult)
                    nc.vector.tensor_tensor(out=lz[:], in0=lz[:], in1=pz[:], op=AL.max)
                    ob = mpool.tile([P, D], F32, tag="ob")
                    nc.vector.tensor_tensor(out=ob[:], in0=lz[:, :D], in1=lz[:, D:], op=AL.add)
                    nc.sync.dma_start(out=out[lo:lo + nn, :], in_=ob[:nn, :])
                full_i += len(g["blocks"])
    nc.compile()
    return nc


# ---------------- PJRT execution ----------------

def _make_exec(nc, device):
    import jax
    from concourse.bass2jax import _bass_exec_p, install_neuronx_cc_hook
    install_neuronx_cc_hook()
    in_names, out_names, out_avals, zero_outs = [], [], [], []
    in_specs = {}
    for alloc in nc.m.functions[0].allocations:
        if not isinstance(alloc, mybir.MemoryLocationSet):
            continue
        name = alloc.memorylocations[0].name
        if alloc.kind == "ExternalInput":
            in_names.append(name)
            in_specs[name] = (tuple(alloc.tensor_shape), mybir.dt.np(alloc.dtype))
        elif alloc.kind == "ExternalOutput":
            out_names.append(name)
            shape = tuple(alloc.tensor_shape)
            dtype = mybir.dt.np(alloc.dtype)
            out_avals.append(jax.core.ShapedArray(shape, dtype))
            zero_outs.append(np.zeros(shape, dtype))
    all_in_names = in_names + out_names

    def _body(*args):
        outs = _bass_exec_p.bind(
            *args,
            out_avals=tuple(out_avals),
            in_names=tuple(all_in_names),
            out_names=tuple(out_names),
            lowering_input_output_aliases=(),
            sim_require_finite=True,
            sim_require_nnan=True,
            nc=nc,
        )
        return tuple(outs)

    jitted = jax.jit(_body, keep_unused=True, device=device)
    return jitted, in_names, out_names, zero_outs, in_specs


class CoreRunner:
    def __init__(self, nc, device, in_map):
        import jax
        self.jax = jax
        (self.jitted, self.in_names, self.out_names, self.zero_outs,
         in_specs) = _make_exec(nc, device)
        self.dev_in = [
            jax.device_put(
                np.asarray(in_map[n]) if n in in_map
                else np.zeros(*in_specs[n][:1], in_specs[n][1]), device)
            for n in self.in_names]
        self.dev_zero = [jax.device_put(z, device) for z in self.zero_outs]

    def run_async(self):
        return self.jitted(*self.dev_in, *self.dev_zero)

    def outputs_np(self):
        outs = self.jax.block_until_ready(self.run_async())
        return {n: np.asarray(o) for n, o in zip(self.out_names, outs)}


# ---------------- top-level entry ----------------

def kernel(ego_embeddings, edge_vals, W1, b1, W2, b2, edge_rows, edge_cols):
    import jax
    ego = np.asarray(ego_embeddings, np.float32)
    edge_vals = np.asarray(edge_vals, np.float32)
    W1 = np.asarray(W1, np.float32); b1 = np.asarray(b1, np.float32)
    W2 = np.asarray(W2, np.float32); b2 = np.asarray(b2, np.float32)
    rows = np.asarray(edge_rows); cols = np.asarray(edge_cols)
    n = ego.shape[0]
    table_bf16 = ego.astype(ml_dtypes.bfloat16)

    bounds = [round(n * c / N_CORES) for c in range(N_CORES + 1)]
    structs = [preprocess_core(rows, cols, edge_vals, bounds[c], bounds[c + 1])
               for c in range(N_CORES)]
    devices = jax.devices()[:N_CORES]

    ncs = [None] * N_CORES
    errs = [None] * N_CORES

    def _build(c):
        try:
            ncs[c] = build_core_program(structs[c], n)
        except Exception as e:  # noqa: BLE001
            errs[c] = e

    threads = [threading.Thread(target=_build, args=(c,)) for c in range(N_CORES)]
    for t in threads:
        t.start()
    for t in threads:
        t.join()
    for e in errs:
        if e is not None:
            raise e

    runners = []
    for c in range(N_CORES):
        in_map = make_core_inputs(structs[c], table_bf16,
                                  ego[bounds[c]:bounds[c + 1]], W1, b1, W2, b2)
        runners.append(CoreRunner(ncs[c], devices[c], in_map))

    global _LAST_RUNNERS, _LAST_NCS
    _LAST_RUNNERS = runners
    _LAST_NCS = ncs
    futs = [r.run_async() for r in runners]
    out = np.empty((n, D), np.float32)
    for c, (r, f) in enumerate(zip(runners, futs)):
        outs = jax.block_until_ready(f)
        out[bounds[c]:bounds[c + 1]] = np.asarray(outs[r.out_names.index("out")])
    return out



# revision 15
# speedup vs baseline: 1.1657x; 1.1657x over previous
"""Trainium2 Bass kernel for the GNN bi-interaction aggregator (v2).

side = segment_sum(ego[edge_cols] * edge_vals, edge_rows)
out  = leaky_relu((ego + side) @ W1.T + b1) + leaky_relu((ego * side) @ W2.T + b2)

Sharding: destination nodes split across 8 NeuronCores; the embedding table
is replicated in fp8e4 (scaled x4) for the edge gather.  Per-core design:
  - SWDGE dma_gather in fp8 (256B rows): descriptor-rate bound, so calls of
    ~4096 descriptors across 4 queues with 3 block-groups in flight.
  - SpMM via fp8 DoubleRow matmuls: 256 edges (2 gather slots) contracted
    per instruction against host-baked sparse selector tiles S (vals x32),
    accumulated into a [128 dest, 256] f32 PSUM block.
  - MLP in fp16: PE transposes pside, DVE forms (ego+side)/(ego*side) in
    transposed layout, two 256-contraction matmuls per branch, leaky-relu
    on the Activation engine (zero-bias fast path), fp16 output.
"""
import sys
import threading

import numpy as np

if "/opt/trn_rl_repo" not in sys.path:
    sys.path.append("/opt/trn_rl_repo")

import ml_dtypes  # noqa: E402
import concourse.bass as bass  # noqa: E402
import concourse.bacc as bacc  # noqa: E402
import concourse.mybir as mybir  # noqa: E402
from concourse.tile import TileContext  # noqa: E402

P = 128
D = 256
N_CORES = 8
CHUNK = 25000
GROUP_BLOCKS = 4
MAX_CALL_SLOTS = 32      # <=4096 idxs per dma_gather call
F32 = mybir.dt.float32
F16 = mybir.dt.float16
BF16 = mybir.dt.bfloat16
FP8 = mybir.dt.float8e4
I16 = mybir.dt.int16
NPFP8 = mybir.dt.np(FP8)
AL = mybir.AluOpType
DR = mybir.MatmulPerfMode.DoubleRow
LRELU = mybir.ActivationFunctionType.Lrelu
NEG_SLOPE = 0.01
VAL_SCALE = 32.0
EGO_SCALE = 4.0
PSUM_SCALE = 1.0 / (VAL_SCALE * EGO_SCALE)
CLASSES = (32, 64, 128)
_LAST_RUNNERS = []
_LAST_NCS = []


# ---------------- host preprocessing ----------------

def _win_class(dmin, dmax, force128=False):
    """Pick (class_width, offset) for dest range [dmin, dmax] within a block.
    Walrus rejects DoubleRow matmuls with a nonzero dst base partition
    (s3d3_mm_valid_dst_partition), so every tile uses the full 128-dest
    window at offset 0."""
    return 128, 0


def preprocess_core(rows, cols, vals, lo, hi):
    """Static gather/tile structure for destination rows [lo, hi)."""
    rows = np.asarray(rows); cols = np.asarray(cols); vals = np.asarray(vals)
    nn = hi - lo
    nb = (nn + P - 1) // P
    sel = (rows >= lo) & (rows < hi)
    r = (rows[sel] - lo).astype(np.int64)
    c = cols[sel].astype(np.int64)
    v = vals[sel].astype(np.float32)
    b = r // P
    ch = c // CHUNK
    d = r - b * P
    order = np.lexsort((d, ch, b))
    r, c, v, b, ch, d = r[order], c[order], v[order], b[order], ch[order], d[order]

    # (b, ch) run boundaries
    key = b * 4 + ch
    starts = np.flatnonzero(np.r_[True, key[1:] != key[:-1]]) if len(key) else np.array([], np.int64)
    ends = np.r_[starts[1:], len(key)] if len(starts) else np.array([], np.int64)
    runs = {}
    for s, e in zip(starts, ends):
        runs[(int(b[s]), int(ch[s]))] = (int(s), int(e))

    groups = []
    idx_cols = []          # per call: [128, nt*8] int16
    icol_total = 0
    # per class: lists of per-slot arrays (t, plane, part_j, drel, val)
    s_t = {w: [] for w in CLASSES}
    s_pl = {w: [] for w in CLASSES}
    s_j = {w: [] for w in CLASSES}
    s_dr = {w: [] for w in CLASSES}
    s_v = {w: [] for w in CLASSES}
    t_count = {w: 0 for w in CLASSES}

    for g0 in range(0, nb, GROUP_BLOCKS):
        gblocks = list(range(g0, min(g0 + GROUP_BLOCKS, nb)))
        slots = []         # (block, dests(np), vals(np))  in gather order
        calls = []         # (ch, s0, nt, icol0)
        g_icol0 = icol_total
        for cc in range(4):
            c_s0 = len(slots)
            flat = []
            for bb in gblocks:
                se = runs.get((bb, cc))
                if se is None:
                    continue
                s, e = se
                for k in range(s, e, P):
                    k2 = min(k + P, e)
                    idx128 = np.full(P, c[k2 - 1], np.int64)
                    idx128[:k2 - k] = c[k:k2]
                    flat.append(idx128)
                    slots.append((bb, d[k:k2], v[k:k2]))
            nt = len(slots) - c_s0
            # split into calls of <= MAX_CALL_SLOTS
            off = 0
            while off < nt:
                take = min(MAX_CALL_SLOTS, nt - off)
                fl = np.concatenate(flat[off:off + take]) - cc * CHUNK
                assert fl.min() >= 0 and fl.max() < CHUNK
                i16 = np.tile(fl.reshape(-1, 16).T.astype(np.int16), (8, 1))
                calls.append((cc, c_s0 + off, take, icol_total))
                idx_cols.append(i16)
                icol_total += i16.shape[1]
                off += take
        g_icols = icol_total - g_icol0

        # pair tiles per block
        blocks = []
        for bb in gblocks:
            bslots = [si for si, (sb, _, _) in enumerate(slots) if sb == bb]
            # contiguous runs of slot ids (per chunk)
            runs_b = []
            for si in bslots:
                if runs_b and runs_b[-1][-1] == si - 1:
                    runs_b[-1].append(si)
                else:
                    runs_b.append([si])
            tiles = []     # [cls, t, s_lo, o, start, stop]
            first = True
            for rb in runs_b:
                k = 0
                while k < len(rb):
                    if k + 1 < len(rb):
                        pls = (rb[k], rb[k + 1]); s_lo = rb[k]; k += 2
                    else:
                        si = rb[k]; k += 1
                        if si > 0:
                            pls = (None, si); s_lo = si - 1
                        else:
                            pls = (si, None); s_lo = si
                    dall = np.concatenate([slots[x][1] for x in pls if x is not None])
                    cls, o = _win_class(int(dall.min()), int(dall.max()), force128=first)
                    assert dall.min() >= o and dall.max() < o + cls
                    t = t_count[cls]; t_count[cls] += 1
                    for plane, x in enumerate(pls):
                        if x is None:
                            continue
                        sd, sv = slots[x][1], slots[x][2]
                        m = len(sd)
                        s_t[cls].append(np.full(m, t, np.int64))
                        s_pl[cls].append(np.full(m, plane, np.int64))
                        s_j[cls].append(np.arange(m, dtype=np.int64))
                        s_dr[cls].append(sd - o)
                        s_v[cls].append(sv)
                    tiles.append([cls, t, s_lo, o, first, False])
                    first = False
            if not tiles:
                # empty block: dummy zero tile for PSUM init
                cls, o = 128, 0
                t = t_count[cls]; t_count[cls] += 1
                tiles.append([cls, t, 0, o, True, False])
            tiles[-1][5] = True
            boff = bb * P
            bn = min(P, nn - boff)
            blocks.append((boff, bn, tiles))
        groups.append(dict(calls=calls, nslots=len(slots), blocks=blocks,
                           icol0=g_icol0, icols=g_icols))

    idx16 = (np.concatenate(idx_cols, axis=1) if idx_cols
             else np.zeros((P, 8), np.int16))

    s_arrays = {}
    for w in CLASSES:
        T = max(t_count[w], 1)
        arr = np.zeros((P, T, 2, w), np.float32)
        if s_t[w]:
            tt = np.concatenate(s_t[w])
            pl = np.concatenate(s_pl[w])
            jj = np.concatenate(s_j[w])
            dr = np.concatenate(s_dr[w])
            vv = np.concatenate(s_v[w])
            arr[jj, tt, pl, dr] = vv * VAL_SCALE
        s_arrays[w] = arr.astype(NPFP8)

    # per-group per-class tile index ranges (tiles were numbered globally in
    # emission order, which is grouped by... NOT contiguous per group).
    # Renumber: per group, per class, count tiles and assign local js.
    t_next = {w: 0 for w in CLASSES}
    for g in groups:
        g_t0 = {w: t_next[w] for w in CLASSES}
        g_n = {w: 0 for w in CLASSES}
        for (_, _, tiles) in g["blocks"]:
            for tl in tiles:
                cls = tl[0]
                g_n[cls] += 1
        for w in CLASSES:
            t_next[w] += g_n[w]
        g["s_t0"] = g_t0
        g["s_n"] = g_n
    # verify global order assumption: tiles of a class are emitted in
    # group-major order by construction of the loops above.
    for w in CLASSES:
        assert t_next[w] == t_count[w]

    max_slots = max(max((g["nslots"] for g in groups), default=2), 2)
    max_icols = max((g["icols"] for g in groups), default=8)
    max_sn = {w: max((g["s_n"][w] for g in groups), default=1) for w in CLASSES}
    return dict(groups=groups, idx16=idx16, s_arrays=s_arrays, nb=nb, nn=nn,
                max_slots=max_slots, max_icols=max_icols, max_sn=max_sn,
                t_count=t_count)


def make_table(ego):
    return (np.asarray(ego, np.float32) * EGO_SCALE).astype(NPFP8)


def make_core_inputs(struct, table_fp8, ego_slice, W1, b1, W2, b2):
    nb = struct["nb"]
    nn = struct["nn"]
    ego_pad = np.zeros((nb * P, D), np.float32)
    ego_pad[:nn] = np.asarray(ego_slice, np.float32)
    egoT = np.ascontiguousarray(
        ego_pad.T.reshape(2, P, nb * P).transpose(1, 0, 2)).astype(np.float16)
    w1t = np.ascontiguousarray(np.asarray(W1, np.float32).T.reshape(2, P, D)).astype(np.float16)
    w2t = np.ascontiguousarray(np.asarray(W2, np.float32).T.reshape(2, P, D)).astype(np.float16)
    m = {
        "table": table_fp8,
        "idx16": struct["idx16"],
        "s32": struct["s_arrays"][32],
        "s64": struct["s_arrays"][64],
        "s128": struct["s_arrays"][128],
        "egoT": egoT,
        "w1t": w1t, "w2t": w2t,
        "ident": np.eye(P, dtype=np.float16),
    }
    b1 = np.asarray(b1, np.float32); b2 = np.asarray(b2, np.float32)
    if b1.any() or b2.any():
        m["b1bc"] = np.tile(b1.reshape(1, D), (P, 1)).astype(np.float32)
        m["b2bc"] = np.tile(b2.reshape(1, D), (P, 1)).astype(np.float32)
    return m


# ---------------- program builder ----------------

def build_core_program(struct, n_table_rows, reps=1, stage='full', has_bias=False,
                       gbufs=3):
    nb = struct["nb"]
    nn = struct["nn"]
    groups = struct["groups"]
    Ti = struct["idx16"].shape[1]
    Ts = {w: struct["s_arrays"][w].shape[1] for w in CLASSES}

    nc = bacc.Bacc("TRN2", target_bir_lowering=False, debug=False, num_swdge_queues=4)
    table = nc.dram_tensor("table", [n_table_rows, D], FP8, kind="ExternalInput")
    idx16 = nc.dram_tensor("idx16", [P, Ti], I16, kind="ExternalInput")
    s_dram = {w: nc.dram_tensor(f"s{w}", [P, Ts[w], 2, w], FP8, kind="ExternalInput")
              for w in CLASSES}
    egoT = nc.dram_tensor("egoT", [P, 2, nb * P], F16, kind="ExternalInput")
    w1t = nc.dram_tensor("w1t", [2, P, D], F16, kind="ExternalInput")
    w2t = nc.dram_tensor("w2t", [2, P, D], F16, kind="ExternalInput")
    ident = nc.dram_tensor("ident", [P, P], F16, kind="ExternalInput")
    if has_bias:
        b1bc = nc.dram_tensor("b1bc", [P, D], F32, kind="ExternalInput")
        b2bc = nc.dram_tensor("b2bc", [P, D], F32, kind="ExternalInput")
    out = nc.dram_tensor("out", [nn, D], F16, kind="ExternalOutput")

    with TileContext(nc) as tc:
        with (
            tc.tile_pool(name="const", bufs=1) as cpool,
            tc.tile_pool(name="g", bufs=gbufs) as gpool,
            tc.tile_pool(name="i", bufs=gbufs) as ipool,
            tc.tile_pool(name="s", bufs=2) as spool,
            tc.tile_pool(name="e", bufs=4) as epool,
            tc.tile_pool(name="m", bufs=3) as mpool,
            tc.tile_pool(name="pside", bufs=2, space="PSUM") as pside_pool,
            tc.tile_pool(name="pt", bufs=2, space="PSUM") as pt_pool,
            tc.tile_pool(name="pz", bufs=2, space="PSUM") as pz_pool,
        ):
            w1t_sb = cpool.tile([P, 2, D], F16)
            nc.sync.dma_start(out=w1t_sb[:], in_=w1t[:, :, :].transpose([1, 0, 2]))
            w2t_sb = cpool.tile([P, 2, D], F16)
            nc.sync.dma_start(out=w2t_sb[:], in_=w2t[:, :, :].transpose([1, 0, 2]))
            ident_sb = cpool.tile([P, P], F16)
            nc.sync.dma_start(out=ident_sb[:], in_=ident[:, :])
            if has_bias:
                b1_sb = cpool.tile([P, D], F32)
                nc.sync.dma_start(out=b1_sb[:], in_=b1bc[:, :])
                b2_sb = cpool.tile([P, D], F32)
                nc.sync.dma_start(out=b2_sb[:], in_=b2bc[:, :])

            qrr = 0
            for _rep in range(reps):
              for g in groups:
                idx_sb = ipool.tile([P, struct["max_icols"]], I16, tag="idx")
                if g["icols"]:
                    nc.sync.dma_start(out=idx_sb[:, :g["icols"]],
                                      in_=idx16[:, g["icol0"]:g["icol0"] + g["icols"]])
                G = gpool.tile([P, struct["max_slots"], D], FP8, tag="G")
                for (cc, s0, nt, icol0) in g["calls"]:
                    li = icol0 - g["icol0"]
                    nidx = nt * P
                    nc.gpsimd.dma_gather(
                        out_ap=G[:, s0:s0 + nt, :],
                        in_ap=table[cc * CHUNK:min((cc + 1) * CHUNK, n_table_rows), :],
                        idxs_ap=idx_sb[:, li:li + nidx // 16],
                        num_idxs=nidx, num_idxs_reg=nidx, elem_size=D,
                        single_packet=False, queue_num=qrr % 4)
                    qrr += 1
                if stage == 'gather':
                    continue
                s_sb = {}
                for w in CLASSES:
                    n_w = g["s_n"][w]
                    if n_w == 0:
                        continue
                    t0 = g["s_t0"][w]
                    s_sb[w] = spool.tile([P, struct["max_sn"][w], 2, w], FP8,
                                         tag=f"s{w}", name=f"s{w}")
                    nc.scalar.dma_start(out=s_sb[w][:, :n_w, :, :],
                                        in_=s_dram[w][:, t0:t0 + n_w, :, :])
                jloc = {w: 0 for w in CLASSES}
                for (boff, bn, tiles) in g["blocks"]:
                    egoT_sb = epool.tile([P, 2, P], F16, tag="egoT")
                    nc.scalar.dma_start(out=egoT_sb[:],
                                        in_=egoT[:, :, boff:boff + P])
                    # full 2KB PSUM bank per buffer: DoubleRow matmuls need a
                    # bank-aligned destination
                    pside = pside_pool.tile([P, 2, D], F32, tag="pside")
                    for (cls, t, s_lo, o, st, sp) in tiles:
                        j = jloc[cls]; jloc[cls] += 1
                        nc.tensor.matmul(
                            out=pside[o:o + cls, 0, :],
                            lhsT=s_sb[cls][:, j, :, :],
                            rhs=G[:, s_lo:s_lo + 2, :],
                            start=st, stop=sp, perf_mode=DR,
                            skip_group_check=True)
                    psideS = mpool.tile([P, D], F16, tag="ps")
                    nc.scalar.mul(out=psideS[:], in_=pside[:, 0, :], mul=PSUM_SCALE)
                    pT = pt_pool.tile([P, 2, P], F16, tag="pT")
                    nc.tensor.transpose(out=pT[:, 0, :], in_=psideS[:, 0:P],
                                        identity=ident_sb[:])
                    nc.tensor.transpose(out=pT[:, 1, :], in_=psideS[:, P:D],
                                        identity=ident_sb[:])
                    sum_inT = mpool.tile([P, 2, P], F16, tag="sum")
                    nc.vector.tensor_tensor(out=sum_inT[:], in0=egoT_sb[:],
                                            in1=pT[:], op=AL.add)
                    bi_inT = mpool.tile([P, 2, P], F16, tag="bi")
                    nc.vector.tensor_tensor(out=bi_inT[:], in0=egoT_sb[:],
                                            in1=pT[:], op=AL.mult)
                    pz = pz_pool.tile([P, 2, D], F32, tag="z")
                    pz1 = pz[:, 0, :]
                    pz2 = pz[:, 1, :]
                    nc.tensor.matmul(out=pz1, lhsT=sum_inT[:, 0, :],
                                     rhs=w1t_sb[:, 0, :], start=True, stop=False,
                                     skip_group_check=True)
                    nc.tensor.matmul(out=pz1, lhsT=sum_inT[:, 1, :],
                                     rhs=w1t_sb[:, 1, :], start=False, stop=True,
                                     skip_group_check=True)
                    nc.tensor.matmul(out=pz2, lhsT=bi_inT[:, 0, :],
                                     rhs=w2t_sb[:, 0, :], start=True, stop=False,
                                     skip_group_check=True)
                    nc.tensor.matmul(out=pz2, lhsT=bi_inT[:, 1, :],
                                     rhs=w2t_sb[:, 1, :], start=False, stop=True,
                                     skip_group_check=True)
                    o1 = mpool.tile([P, D], F16, tag="o1")
                    o2 = mpool.tile([P, D], F16, tag="o2")
                    if has_bias:
                        t1 = mpool.tile([P, D], F32, tag="t1")
                        nc.vector.tensor_tensor(out=t1[:], in0=pz1, in1=b1_sb[:], op=AL.add)
                        t1m = mpool.tile([P, D], F32, tag="t1m")
                        nc.vector.tensor_scalar(out=t1m[:], in0=t1[:], scalar1=NEG_SLOPE,
                                                scalar2=None, op0=AL.mult)
                        nc.vector.tensor_tensor(out=o1[:], in0=t1[:], in1=t1m[:], op=AL.max)
                        t2 = mpool.tile([P, D], F32, tag="t2")
                        nc.vector.tensor_tensor(out=t2[:], in0=pz2, in1=b2_sb[:], op=AL.add)
                        t2m = mpool.tile([P, D], F32, tag="t2m")
                        nc.vector.tensor_scalar(out=t2m[:], in0=t2[:], scalar1=NEG_SLOPE,
                                                scalar2=None, op0=AL.mult)
                        nc.vector.tensor_tensor(out=o2[:], in0=t2[:], in1=t2m[:], op=AL.max)
                    else:
                        nc.scalar.activation(out=o1[:], in_=pz1, func=LRELU,
                                             alpha=NEG_SLOPE)
                        nc.scalar.activation(out=o2[:], in_=pz2, func=LRELU,
                                             alpha=NEG_SLOPE)
                    ob = mpool.tile([P, D], F16, tag="ob")
                    nc.vector.tensor_tensor(out=ob[:], in0=o1[:], in1=o2[:], op=AL.add)
                    nc.sync.dma_start(out=out[boff:boff + bn, :], in_=ob[:bn, :])
    nc.compile()
    return nc


# ---------------- PJRT execution ----------------

def _make_exec(nc, device):
    import jax
    from concourse.bass2jax import _bass_exec_p, install_neuronx_cc_hook
    install_neuronx_cc_hook()
    in_names, out_names, out_avals, zero_outs = [], [], [], []
    in_specs = {}
    for alloc in nc.m.functions[0].allocations:
        if not isinstance(alloc, mybir.MemoryLocationSet):
            continue
        name = alloc.memorylocations[0].name
        if alloc.kind == "ExternalInput":
            in_names.append(name)
            in_specs[name] = (tuple(alloc.tensor_shape), mybir.dt.np(alloc.dtype))
        elif alloc.kind == "ExternalOutput":
            out_names.append(name)
            shape = tuple(alloc.tensor_shape)
            dtype = mybir.dt.np(alloc.dtype)
            out_avals.append(jax.core.ShapedArray(shape, dtype))
            zero_outs.append(np.zeros(shape, dtype))
    all_in_names = in_names + out_names

    def _body(*args):
        outs = _bass_exec_p.bind(
            *args,
            out_avals=tuple(out_avals),
            in_names=tuple(all_in_names),
            out_names=tuple(out_names),
            lowering_input_output_aliases=(),
            sim_require_finite=True,
            sim_require_nnan=True,
            nc=nc,
        )
        return tuple(outs)

    jitted = jax.jit(_body, keep_unused=True, device=device)
    return jitted, in_names, out_names, zero_outs, in_specs


class CoreRunner:
    def __init__(self, nc, device, in_map):
        import jax
        self.jax = jax
        (self.jitted, self.in_names, self.out_names, self.zero_outs,
         in_specs) = _make_exec(nc, device)
        self.dev_in = [
            jax.device_put(
                np.asarray(in_map[n]) if n in in_map
                else np.zeros(*in_specs[n][:1], in_specs[n][1]), device)
            for n in self.in_names]
        self.dev_zero = [jax.device_put(z, device) for z in self.zero_outs]

    def run_async(self):
        return self.jitted(*self.dev_in, *self.dev_zero)

    def outputs_np(self):
        outs = self.jax.block_until_ready(self.run_async())
        return {n: np.asarray(o) for n, o in zip(self.out_names, outs)}


# ---------------- top-level entry ----------------

def kernel(ego_embeddings, edge_vals, W1, b1, W2, b2, edge_rows, edge_cols):
    import jax
    ego = np.asarray(ego_embeddings, np.float32)
    edge_vals = np.asarray(edge_vals, np.float32)
    W1 = np.asarray(W1, np.float32); b1 = np.asarray(b1, np.float32)
    W2 = np.asarray(W2, np.float32); b2 = np.asarray(b2, np.float32)
    rows = np.asarray(edge_rows); cols = np.asarray(edge_cols)
    n = ego.shape[0]
    table_fp8 = make_table(ego)
    has_bias = bool(b1.any() or b2.any())

    bounds = [round(n * c / N_CORES) for c in range(N_CORES + 1)]
    structs = [None] * N_CORES
    ncs = [None] * N_CORES
    errs = [None] * N_CORES

    def _build(c):
        try:
            structs[c] = preprocess_core(rows, cols, edge_vals,
                                         bounds[c], bounds[c + 1])
            ncs[c] = build_core_program(structs[c], n, has_bias=has_bias)
        except Exception as e:  # noqa: BLE001
            errs[c] = e

    threads = [threading.Thread(target=_build, args=(c,)) for c in range(N_CORES)]
    for t in threads:
        t.start()
    for t in threads:
        t.join()
    for e in errs:
        if e is not None:
            raise e

    devices = jax.devices()[:N_CORES]
    runners = []
    for c in range(N_CORES):
        in_map = make_core_inputs(structs[c], table_fp8,
                                  ego[bounds[c]:bounds[c + 1]], W1, b1, W2, b2)
        runners.append(CoreRunner(ncs[c], devices[c], in_map))

    global _LAST_RUNNERS, _LAST_NCS
    _LAST_RUNNERS = runners
    _LAST_NCS = ncs
    futs = [r.run_async() for r in runners]
    out = np.empty((n, D), np.float32)
    for c, (r, f) in enumerate(zip(runners, futs)):
        outs = jax.block_until_ready(f)
        out[bounds[c]:bounds[c + 1]] = np.asarray(
            outs[r.out_names.index("out")], ).astype(np.float32)
    return out


# revision 23
# speedup vs baseline: 1.9528x; 1.6752x over previous
"""Trainium2 Bass kernel for the GNN bi-interaction aggregator (v2).

side = segment_sum(ego[edge_cols] * edge_vals, edge_rows)
out  = leaky_relu((ego + side) @ W1.T + b1) + leaky_relu((ego * side) @ W2.T + b2)

Sharding: destination nodes split across 8 NeuronCores; the embedding table
is replicated in fp8e4 (scaled x4) for the edge gather.  Per-core design:
  - SWDGE dma_gather in fp8 (256B rows): descriptor-rate bound, so calls of
    ~4096 descriptors across 4 queues with 3 block-groups in flight.
  - SpMM via fp8 DoubleRow matmuls: 256 edges (2 gather slots) contracted
    per instruction against host-baked sparse selector tiles S (vals x32),
    accumulated into a [128 dest, 256] f32 PSUM block.
  - MLP in fp16: PE transposes pside, DVE forms (ego+side)/(ego*side) in
    transposed layout, two 256-contraction matmuls per branch, leaky-relu
    on the Activation engine (zero-bias fast path), fp16 output.
"""
import sys
import threading

import numpy as np

if "/opt/trn_rl_repo" not in sys.path:
    sys.path.append("/opt/trn_rl_repo")

import ml_dtypes  # noqa: E402
import concourse.bass as bass  # noqa: E402
import concourse.bacc as bacc  # noqa: E402
import concourse.mybir as mybir  # noqa: E402
from concourse.tile import TileContext  # noqa: E402

P = 128
D = 256
N_CORES = 8
CHUNK = 25000
GROUP_BLOCKS = 4
MAX_CALL_SLOTS = 32      # <=4096 idxs per dma_gather call
F32 = mybir.dt.float32
F16 = mybir.dt.float16
BF16 = mybir.dt.bfloat16
FP8 = mybir.dt.float8e4
I16 = mybir.dt.int16
NPFP8 = mybir.dt.np(FP8)
AL = mybir.AluOpType
DR = mybir.MatmulPerfMode.DoubleRow
LRELU = mybir.ActivationFunctionType.Lrelu
NEG_SLOPE = 0.01
VAL_SCALE = 32.0
EGO_SCALE = 4.0
PSUM_SCALE = 1.0 / (VAL_SCALE * EGO_SCALE)
CLASSES = (32, 64, 128)
_LAST_RUNNERS = []
_LAST_NCS = []


# ---------------- host preprocessing ----------------

def _win_class(dmin, dmax, force128=False):
    """Pick (class_width, offset) for dest range [dmin, dmax] within a block.
    Walrus rejects DoubleRow matmuls with a nonzero dst base partition
    (s3d3_mm_valid_dst_partition), so every tile uses the full 128-dest
    window at offset 0."""
    return 128, 0


def preprocess_core(rows, cols, vals, lo, hi):
    """Static gather/tile structure for destination rows [lo, hi)."""
    rows = np.asarray(rows); cols = np.asarray(cols); vals = np.asarray(vals)
    nn = hi - lo
    nb = (nn + P - 1) // P
    sel = (rows >= lo) & (rows < hi)
    r = (rows[sel] - lo).astype(np.int64)
    c = cols[sel].astype(np.int64)
    v = vals[sel].astype(np.float32)
    b = r // P
    ch = c // CHUNK
    d = r - b * P
    order = np.lexsort((d, ch, b))
    r, c, v, b, ch, d = r[order], c[order], v[order], b[order], ch[order], d[order]

    # (b, ch) run boundaries
    key = b * 4 + ch
    starts = np.flatnonzero(np.r_[True, key[1:] != key[:-1]]) if len(key) else np.array([], np.int64)
    ends = np.r_[starts[1:], len(key)] if len(starts) else np.array([], np.int64)
    runs = {}
    for s, e in zip(starts, ends):
        runs[(int(b[s]), int(ch[s]))] = (int(s), int(e))

    groups = []
    idx_cols = []          # per call: [128, nt*8] int16
    icol_total = 0
    # per class: lists of per-slot arrays (t, plane, part_j, drel, val)
    s_t = {w: [] for w in CLASSES}
    s_pl = {w: [] for w in CLASSES}
    s_j = {w: [] for w in CLASSES}
    s_dr = {w: [] for w in CLASSES}
    s_v = {w: [] for w in CLASSES}
    t_count = {w: 0 for w in CLASSES}

    for g0 in range(0, nb, GROUP_BLOCKS):
        gblocks = list(range(g0, min(g0 + GROUP_BLOCKS, nb)))
        slots = []         # (block, dests(np), vals(np))  in gather order
        calls = []         # (ch, s0, nt, icol0)
        g_icol0 = icol_total
        for cc in range(4):
            c_s0 = len(slots)
            flat = []
            run_lens = []       # slots per (block, cc) run, in order
            for bb in gblocks:
                se = runs.get((bb, cc))
                if se is None:
                    continue
                s, e = se
                nsl = 0
                for k in range(s, e, P):
                    k2 = min(k + P, e)
                    idx128 = np.full(P, c[k2 - 1], np.int64)
                    idx128[:k2 - k] = c[k:k2]
                    flat.append(idx128)
                    slots.append((bb, d[k:k2], v[k:k2]))
                    nsl += 1
                run_lens.append(nsl)
            nt = len(slots) - c_s0
            if nt == 0:
                continue
            # pack whole runs into balanced calls of <= MAX_CALL_SLOTS slots
            # (runs stay intact so slot pairs never straddle a call)
            ncalls_cc = -(-nt // MAX_CALL_SLOTS)
            target = nt / ncalls_cc
            sizes = []
            cur = 0
            for rl in run_lens:
                if cur > 0 and (cur + rl > MAX_CALL_SLOTS or cur >= target):
                    sizes.append(cur)
                    cur = 0
                cur += rl
            if cur:
                sizes.append(cur)
            off = 0
            for take in sizes:
                fl = np.concatenate(flat[off:off + take]) - cc * CHUNK
                assert fl.min() >= 0 and fl.max() < CHUNK
                i16 = np.tile(fl.reshape(-1, 16).T.astype(np.int16), (8, 1))
                calls.append([cc, c_s0 + off, take, icol_total, 0])
                idx_cols.append(i16)
                icol_total += i16.shape[1]
                off += take
        g_icols = icol_total - g_icol0
        # queue assignment: least-loaded (by descriptors) within the group
        qload = [0, 0, 0, 0]
        for call in sorted(calls, key=lambda cl: -cl[2]):
            q = min(range(4), key=lambda x: qload[x])
            qload[q] += call[2]
            call[4] = q

        # pair tiles per block
        blocks = []
        for bb in gblocks:
            bslots = [si for si, (sb, _, _) in enumerate(slots) if sb == bb]
            # contiguous runs of slot ids (per chunk)
            runs_b = []
            for si in bslots:
                if runs_b and runs_b[-1][-1] == si - 1:
                    runs_b[-1].append(si)
                else:
                    runs_b.append([si])
            tiles = []     # [cls, t, s_lo, o, start, stop]
            first = True
            for rb in runs_b:
                k = 0
                while k < len(rb):
                    if k + 1 < len(rb):
                        pls = (rb[k], rb[k + 1]); s_lo = rb[k]; k += 2
                    else:
                        si = rb[k]; k += 1
                        if si > 0:
                            pls = (None, si); s_lo = si - 1
                        else:
                            pls = (si, None); s_lo = si
                    dall = np.concatenate([slots[x][1] for x in pls if x is not None])
                    cls, o = _win_class(int(dall.min()), int(dall.max()), force128=first)
                    assert dall.min() >= o and dall.max() < o + cls
                    t = t_count[cls]; t_count[cls] += 1
                    for plane, x in enumerate(pls):
                        if x is None:
                            continue
                        sd, sv = slots[x][1], slots[x][2]
                        m = len(sd)
                        s_t[cls].append(np.full(m, t, np.int64))
                        s_pl[cls].append(np.full(m, plane, np.int64))
                        s_j[cls].append(np.arange(m, dtype=np.int64))
                        s_dr[cls].append(sd - o)
                        s_v[cls].append(sv)
                    tiles.append([cls, t, s_lo, o, first, False])
                    first = False
            if not tiles:
                # empty block: dummy zero tile for PSUM init
                cls, o = 128, 0
                t = t_count[cls]; t_count[cls] += 1
                tiles.append([cls, t, 0, o, True, False])
            tiles[-1][5] = True
            boff = bb * P
            bn = min(P, nn - boff)
            blocks.append((boff, bn, tiles))
        groups.append(dict(calls=calls, nslots=len(slots), blocks=blocks,
                           icol0=g_icol0, icols=g_icols))

    idx16 = (np.concatenate(idx_cols, axis=1) if idx_cols
             else np.zeros((P, 8), np.int16))

    s_arrays = {}
    for w in CLASSES:
        T = max(t_count[w], 1)
        arr = np.zeros((P, T, 2, w), np.float32)
        if s_t[w]:
            tt = np.concatenate(s_t[w])
            pl = np.concatenate(s_pl[w])
            jj = np.concatenate(s_j[w])
            dr = np.concatenate(s_dr[w])
            vv = np.concatenate(s_v[w])
            arr[jj, tt, pl, dr] = vv * VAL_SCALE
        s_arrays[w] = arr.astype(NPFP8)

    # per-group per-class tile index ranges (tiles were numbered globally in
    # emission order, which is grouped by... NOT contiguous per group).
    # Renumber: per group, per class, count tiles and assign local js.
    t_next = {w: 0 for w in CLASSES}
    for g in groups:
        g_t0 = {w: t_next[w] for w in CLASSES}
        g_n = {w: 0 for w in CLASSES}
        for (_, _, tiles) in g["blocks"]:
            for tl in tiles:
                cls = tl[0]
                g_n[cls] += 1
        for w in CLASSES:
            t_next[w] += g_n[w]
        g["s_t0"] = g_t0
        g["s_n"] = g_n
    # verify global order assumption: tiles of a class are emitted in
    # group-major order by construction of the loops above.
    for w in CLASSES:
        assert t_next[w] == t_count[w]

    max_slots = max(max((g["nslots"] for g in groups), default=2), 2)
    max_icols = max((g["icols"] for g in groups), default=8)
    max_sn = {w: max((g["s_n"][w] for g in groups), default=1) for w in CLASSES}
    return dict(groups=groups, idx16=idx16, s_arrays=s_arrays, nb=nb, nn=nn,
                max_slots=max_slots, max_icols=max_icols, max_sn=max_sn,
                t_count=t_count)


def make_table(ego):
    return (np.asarray(ego, np.float32) * EGO_SCALE).astype(NPFP8)


def make_core_inputs(struct, table_fp8, ego_slice, W1, b1, W2, b2):
    nb = struct["nb"]
    nn = struct["nn"]
    ego_pad = np.zeros((nb * P, D), np.float32)
    ego_pad[:nn] = np.asarray(ego_slice, np.float32)
    egoT = np.ascontiguousarray(
        ego_pad.T.reshape(2, P, nb * P).transpose(1, 0, 2)).astype(np.float16)
    w1t = np.ascontiguousarray(np.asarray(W1, np.float32).T.reshape(2, P, D)).astype(np.float16)
    w2t = np.ascontiguousarray(np.asarray(W2, np.float32).T.reshape(2, P, D)).astype(np.float16)
    m = {
        "table": table_fp8,
        "idx16": struct["idx16"],
        "s32": struct["s_arrays"][32],
        "s64": struct["s_arrays"][64],
        "s128": struct["s_arrays"][128],
        "egoT": egoT,
        "w1t": w1t, "w2t": w2t,
        "ident": np.eye(P, dtype=np.float16),
    }
    b1 = np.asarray(b1, np.float32); b2 = np.asarray(b2, np.float32)
    if b1.any() or b2.any():
        m["b1bc"] = np.tile(b1.reshape(1, D), (P, 1)).astype(np.float32)
        m["b2bc"] = np.tile(b2.reshape(1, D), (P, 1)).astype(np.float32)
    return m


# ---------------- program builder ----------------

def build_core_program(struct, n_table_rows, reps=1, stage='full', has_bias=False,
                       gbufs=3, sbufs=2, s_on_sp=True):
    nb = struct["nb"]
    nn = struct["nn"]
    groups = struct["groups"]
    Ti = struct["idx16"].shape[1]
    Ts = {w: struct["s_arrays"][w].shape[1] for w in CLASSES}

    nc = bacc.Bacc("TRN2", target_bir_lowering=False, debug=False, num_swdge_queues=4)
    table = nc.dram_tensor("table", [n_table_rows, D], FP8, kind="ExternalInput")
    idx16 = nc.dram_tensor("idx16", [P, Ti], I16, kind="ExternalInput")
    s_dram = {w: nc.dram_tensor(f"s{w}", [P, Ts[w], 2, w], FP8, kind="ExternalInput")
              for w in CLASSES}
    egoT = nc.dram_tensor("egoT", [P, 2, nb * P], F16, kind="ExternalInput")
    w1t = nc.dram_tensor("w1t", [2, P, D], F16, kind="ExternalInput")
    w2t = nc.dram_tensor("w2t", [2, P, D], F16, kind="ExternalInput")
    ident = nc.dram_tensor("ident", [P, P], F16, kind="ExternalInput")
    if has_bias:
        b1bc = nc.dram_tensor("b1bc", [P, D], F32, kind="ExternalInput")
        b2bc = nc.dram_tensor("b2bc", [P, D], F32, kind="ExternalInput")
    out = nc.dram_tensor("out", [nn, D], F16, kind="ExternalOutput")

    with TileContext(nc) as tc:
        with (
            tc.tile_pool(name="const", bufs=1) as cpool,
            tc.tile_pool(name="g", bufs=gbufs) as gpool,
            tc.tile_pool(name="i", bufs=gbufs) as ipool,
            tc.tile_pool(name="s", bufs=sbufs) as spool,
            tc.tile_pool(name="e", bufs=4) as epool,
            tc.tile_pool(name="m", bufs=3) as mpool,
            tc.tile_pool(name="pside", bufs=2, space="PSUM") as pside_pool,
            tc.tile_pool(name="pt", bufs=2, space="PSUM") as pt_pool,
            tc.tile_pool(name="pz", bufs=2, space="PSUM") as pz_pool,
        ):
            w1t_sb = cpool.tile([P, 2, D], F16)
            nc.sync.dma_start(out=w1t_sb[:], in_=w1t[:, :, :].transpose([1, 0, 2]))
            w2t_sb = cpool.tile([P, 2, D], F16)
            nc.sync.dma_start(out=w2t_sb[:], in_=w2t[:, :, :].transpose([1, 0, 2]))
            ident_sb = cpool.tile([P, P], F16)
            nc.sync.dma_start(out=ident_sb[:], in_=ident[:, :])
            if has_bias:
                b1_sb = cpool.tile([P, D], F32)
                nc.sync.dma_start(out=b1_sb[:], in_=b1bc[:, :])
                b2_sb = cpool.tile([P, D], F32)
                nc.sync.dma_start(out=b2_sb[:], in_=b2bc[:, :])

            qrr = 0
            for _rep in range(reps):
              for g in groups:
                idx_sb = ipool.tile([P, struct["max_icols"]], I16, tag="idx")
                if g["icols"]:
                    nc.sync.dma_start(out=idx_sb[:, :g["icols"]],
                                      in_=idx16[:, g["icol0"]:g["icol0"] + g["icols"]])
                G = gpool.tile([P, struct["max_slots"], D], FP8, tag="G")
                for (cc, s0, nt, icol0, qn) in g["calls"]:
                    li = icol0 - g["icol0"]
                    nidx = nt * P
                    nc.gpsimd.dma_gather(
                        out_ap=G[:, s0:s0 + nt, :],
                        in_ap=table[cc * CHUNK:min((cc + 1) * CHUNK, n_table_rows), :],
                        idxs_ap=idx_sb[:, li:li + nidx // 16],
                        num_idxs=nidx, num_idxs_reg=nidx, elem_size=D,
                        single_packet=False, queue_num=qn)
                    qrr += 1
                if stage == 'gather':
                    continue
                s_sb = {}
                _skip_blocks = stage == 'gs'
                for w in CLASSES:
                    n_w = g["s_n"][w]
                    if n_w == 0:
                        continue
                    t0 = g["s_t0"][w]
                    s_sb[w] = spool.tile([P, struct["max_sn"][w], 2, w], FP8,
                                         tag=f"s{w}", name=f"s{w}")
                    s_eng = nc.sync if s_on_sp else nc.scalar
                    s_eng.dma_start(out=s_sb[w][:, :n_w, :, :],
                                    in_=s_dram[w][:, t0:t0 + n_w, :, :])
                if _skip_blocks:
                    continue
                jloc = {w: 0 for w in CLASSES}
                for (boff, bn, tiles) in g["blocks"]:
                    egoT_sb = epool.tile([P, 2, P], F16, tag="egoT")
                    nc.scalar.dma_start(out=egoT_sb[:],
                                        in_=egoT[:, :, boff:boff + P])
                    # full 2KB PSUM bank per buffer: DoubleRow matmuls need a
                    # bank-aligned destination
                    pside = pside_pool.tile([P, 2, D], F32, tag="pside")
                    for (cls, t, s_lo, o, st, sp) in tiles:
                        j = jloc[cls]; jloc[cls] += 1
                        nc.tensor.matmul(
                            out=pside[o:o + cls, 0, :],
                            lhsT=s_sb[cls][:, j, :, :],
                            rhs=G[:, s_lo:s_lo + 2, :],
                            start=st, stop=sp, perf_mode=DR,
                            skip_group_check=True)
                    psideS = mpool.tile([P, D], F16, tag="ps")
                    nc.scalar.mul(out=psideS[:], in_=pside[:, 0, :], mul=PSUM_SCALE)
                    pT = pt_pool.tile([P, 2, P], F16, tag="pT")
                    nc.tensor.transpose(out=pT[:, 0, :], in_=psideS[:, 0:P],
                                        identity=ident_sb[:])
                    nc.tensor.transpose(out=pT[:, 1, :], in_=psideS[:, P:D],
                                        identity=ident_sb[:])
                    sum_inT = mpool.tile([P, 2, P], F16, tag="sum")
                    nc.vector.tensor_tensor(out=sum_inT[:], in0=egoT_sb[:],
                                            in1=pT[:], op=AL.add)
                    bi_inT = mpool.tile([P, 2, P], F16, tag="bi")
                    nc.vector.tensor_tensor(out=bi_inT[:], in0=egoT_sb[:],
                                            in1=pT[:], op=AL.mult)
                    pz = pz_pool.tile([P, 2, D], F32, tag="z")
                    pz1 = pz[:, 0, :]
                    pz2 = pz[:, 1, :]
                    nc.tensor.matmul(out=pz1, lhsT=sum_inT[:, 0, :],
                                     rhs=w1t_sb[:, 0, :], start=True, stop=False,
                                     skip_group_check=True)
                    nc.tensor.matmul(out=pz1, lhsT=sum_inT[:, 1, :],
                                     rhs=w1t_sb[:, 1, :], start=False, stop=True,
                                     skip_group_check=True)
                    nc.tensor.matmul(out=pz2, lhsT=bi_inT[:, 0, :],
                                     rhs=w2t_sb[:, 0, :], start=True, stop=False,
                                     skip_group_check=True)
                    nc.tensor.matmul(out=pz2, lhsT=bi_inT[:, 1, :],
                                     rhs=w2t_sb[:, 1, :], start=False, stop=True,
                                     skip_group_check=True)
                    o1 = mpool.tile([P, D], F16, tag="o1")
                    o2 = mpool.tile([P, D], F16, tag="o2")
                    if has_bias:
                        t1 = mpool.tile([P, D], F32, tag="t1")
                        nc.vector.tensor_tensor(out=t1[:], in0=pz1, in1=b1_sb[:], op=AL.add)
                        t1m = mpool.tile([P, D], F32, tag="t1m")
                        nc.vector.tensor_scalar(out=t1m[:], in0=t1[:], scalar1=NEG_SLOPE,
                                                scalar2=None, op0=AL.mult)
                        nc.vector.tensor_tensor(out=o1[:], in0=t1[:], in1=t1m[:], op=AL.max)
                        t2 = mpool.tile([P, D], F32, tag="t2")
                        nc.vector.tensor_tensor(out=t2[:], in0=pz2, in1=b2_sb[:], op=AL.add)
                        t2m = mpool.tile([P, D], F32, tag="t2m")
                        nc.vector.tensor_scalar(out=t2m[:], in0=t2[:], scalar1=NEG_SLOPE,
                                                scalar2=None, op0=AL.mult)
                        nc.vector.tensor_tensor(out=o2[:], in0=t2[:], in1=t2m[:], op=AL.max)
                    else:
                        nc.scalar.activation(out=o1[:], in_=pz1, func=LRELU,
                                             alpha=NEG_SLOPE)
                        nc.scalar.activation(out=o2[:], in_=pz2, func=LRELU,
                                             alpha=NEG_SLOPE)
                    ob = mpool.tile([P, D], F16, tag="ob")
                    nc.vector.tensor_tensor(out=ob[:], in0=o1[:], in1=o2[:], op=AL.add)
                    nc.sync.dma_start(out=out[boff:boff + bn, :], in_=ob[:bn, :])
    nc.compile()
    return nc


# ---------------- PJRT execution ----------------

def _make_exec(nc, device):
    import jax
    from concourse.bass2jax import _bass_exec_p, install_neuronx_cc_hook
    install_neuronx_cc_hook()
    in_names, out_names, out_avals, zero_outs = [], [], [], []
    in_specs = {}
    for alloc in nc.m.functions[0].allocations:
        if not isinstance(alloc, mybir.MemoryLocationSet):
            continue
        name = alloc.memorylocations[0].name
        if alloc.kind == "ExternalInput":
            in_names.append(name)
            in_specs[name] = (tuple(alloc.tensor_shape), mybir.dt.np(alloc.dtype))
        elif alloc.kind == "ExternalOutput":
            out_names.append(name)
            shape = tuple(alloc.tensor_shape)
            dtype = mybir.dt.np(alloc.dtype)
            out_avals.append(jax.core.ShapedArray(shape, dtype))
            zero_outs.append(np.zeros(shape, dtype))
    all_in_names = in_names + out_names

    def _body(*args):
        outs = _bass_exec_p.bind(
            *args,
            out_avals=tuple(out_avals),
            in_names=tuple(all_in_names),
            out_names=tuple(out_names),
            lowering_input_output_aliases=(),
            sim_require_finite=True,
            sim_require_nnan=True,
            nc=nc,
        )
        return tuple(outs)

    jitted = jax.jit(_body, keep_unused=True, device=device)
    return jitted, in_names, out_names, zero_outs, in_specs


class CoreRunner:
    def __init__(self, nc, device, in_map):
        import jax
        self.jax = jax
        (self.jitted, self.in_names, self.out_names, self.zero_outs,
         in_specs) = _make_exec(nc, device)
        self.dev_in = [
            jax.device_put(
                np.asarray(in_map[n]) if n in in_map
                else np.zeros(*in_specs[n][:1], in_specs[n][1]), device)
            for n in self.in_names]
        self.dev_zero = [jax.device_put(z, device) for z in self.zero_outs]

    def run_async(self):
        return self.jitted(*self.dev_in, *self.dev_zero)

    def outputs_np(self):
        outs = self.jax.block_until_ready(self.run_async())
        return {n: np.asarray(o) for n, o in zip(self.out_names, outs)}


# ---------------- top-level entry ----------------

def kernel(ego_embeddings, edge_vals, W1, b1, W2, b2, edge_rows, edge_cols):
    import jax
    ego = np.asarray(ego_embeddings, np.float32)
    edge_vals = np.asarray(edge_vals, np.float32)
    W1 = np.asarray(W1, np.float32); b1 = np.asarray(b1, np.float32)
    W2 = np.asarray(W2, np.float32); b2 = np.asarray(b2, np.float32)
    rows = np.asarray(edge_rows); cols = np.asarray(edge_cols)
    n = ego.shape[0]
    table_fp8 = make_table(ego)
    has_bias = bool(b1.any() or b2.any())

    bounds = [round(n * c / N_CORES) for c in range(N_CORES + 1)]
    structs = [None] * N_CORES
    ncs = [None] * N_CORES
    errs = [None] * N_CORES

    def _build(c):
        try:
            structs[c] = preprocess_core(rows, cols, edge_vals,
                                         bounds[c], bounds[c + 1])
            ncs[c] = build_core_program(structs[c], n, has_bias=has_bias)
        except Exception as e:  # noqa: BLE001
            errs[c] = e

    threads = [threading.Thread(target=_build, args=(c,)) for c in range(N_CORES)]
    for t in threads:
        t.start()
    for t in threads:
        t.join()
    for e in errs:
        if e is not None:
            raise e

    devices = jax.devices()[:N_CORES]
    runners = []
    for c in range(N_CORES):
        in_map = make_core_inputs(structs[c], table_fp8,
                                  ego[bounds[c]:bounds[c + 1]], W1, b1, W2, b2)
        runners.append(CoreRunner(ncs[c], devices[c], in_map))

    global _LAST_RUNNERS, _LAST_NCS
    _LAST_RUNNERS = runners
    _LAST_NCS = ncs
    futs = [r.run_async() for r in runners]
    out = np.empty((n, D), np.float32)
    for c, (r, f) in enumerate(zip(runners, futs)):
        outs = jax.block_until_ready(f)
        out[bounds[c]:bounds[c + 1]] = np.asarray(
            outs[r.out_names.index("out")], ).astype(np.float32)
    return out
